# revision 15
# baseline (speedup 1.0000x reference)
"""Multi-head attention (B=4, S=2048, E=1024, H=16, causal) on 8 Trainium2 cores.

Sharding: core = (batch b, head-group g) — 4 batches x 2 groups of 8 heads.
Each core computes q/k/v projections for its batch restricted to its 8 heads,
causal attention for those heads, and a partial output projection over its
512 ctx columns.  The host sums the two partials per batch and adds all
output-side bias terms analytically (softmax rows sum to one, so the v-bias
passes through attention unchanged: out += o_b + v_b @ o_w.T).

On-device layouts (per core):
  qT/kT: [head_dim 512 -> 4 tiles of 128, token 2048]  (2 heads per tile)
  v_aug: [token -> 16 tiles of 128, 8 heads x (64 dims + ones col)]
  scores are computed transposed (k^T q per head, contraction dim 64,
  two heads row-tiled concurrently in the PE array), softmax is max-free
  (scores are O(+-8), exp cannot overflow fp32), causal masking is
  structural: fully-masked tiles are skipped, diagonal tiles restrict the
  matmul N-range to the valid query span and a 128-wide affine_select
  staircase zeroes the in-band upper triangle.
  attn @ v is computed as v_aug^T @ expT giving ctx^T plus the softmax
  row-sum in one matmul (ones column of v_aug).

Scheduling structure (v3):
  The kernel is globally near-balanced (PE ~200us streaming, ScalarE exp
  ~180us), but causal attention concentrates exp work in the last query
  blocks.  To keep both engines busy everywhere — and to keep the PE duty
  cycle high enough that the HAM clock gate stays at full rate — each
  attention block j>=1 is split:
    body  = k-tiles 0..4j-1 (needs only qT(j) and earlier kT/v blocks)
    diag  = k-tiles 4j..4j+3 (needs kT/v of block j)
  All four q-projections run up front, so bodies of late blocks overlap
  the k/v projections of early blocks.  Body context sums are evacuated
  to SBUF (bf16) and the diagonal contribution is added in on the DVE.
  Dedicated PSUM pools (scores 2x[128,1024], ctx 2x[68,512],
  proj/oproj/norm 2x[128,512]) keep the phases from serializing on banks.
  Output fT is bf16 (halves the output DMA; host accumulates in f32).
"""

import os
import sys

for _p in ("/opt/trn_rl_repo", "/root/.axon_site/_ro/trn_rl_repo"):
    if os.path.isdir(_p) and _p not in sys.path:
        sys.path.append(_p)

import numpy as np
import ml_dtypes

import concourse.bacc as bacc
import concourse.mybir as mybir
from concourse import tile
from concourse import bass_utils
from concourse.bass import ts

BF16 = ml_dtypes.bfloat16
F32 = mybir.dt.float32
BF = mybir.dt.bfloat16
AFT = mybir.ActivationFunctionType
ALU = mybir.AluOpType

B, S, E = 4, 2048, 1024
H, D = 16, 64
G = 512            # head dims per core (8 heads)
KC = E // 128      # contraction chunks for projections
NM = G // 128      # m-tiles of the group dim
NJ = S // 512      # 512-wide token column blocks
NT = S // 128      # 128-wide token tiles

_NC = None


def _build():
    nc = bacc.Bacc("TRN2", target_bir_lowering=False, debug=False, num_devices=8)

    xq = nc.dram_tensor("xq", (E, S), BF, kind="ExternalInput").ap()
    xk = nc.dram_tensor("xk", (E, S), BF, kind="ExternalInput").ap()
    xv = nc.dram_tensor("xv", (E, S), BF, kind="ExternalInput").ap()
    wq = nc.dram_tensor("wq", (E, G), BF, kind="ExternalInput").ap()
    wk = nc.dram_tensor("wk", (E, G), BF, kind="ExternalInput").ap()
    wv = nc.dram_tensor("wv", (E, G), BF, kind="ExternalInput").ap()
    wo = nc.dram_tensor("wo", (G, E), BF, kind="ExternalInput").ap()
    qb = nc.dram_tensor("qb", (128, NM), F32, kind="ExternalInput").ap()
    kb = nc.dram_tensor("kb", (128, NM), F32, kind="ExternalInput").ap()
    sel = nc.dram_tensor("sel", (4, G), BF, kind="ExternalInput").ap()
    fT = nc.dram_tensor("fT", (E, S), BF, kind="ExternalOutput").ap()

    with tile.TileContext(nc) as tc:
        with (
            tc.tile_pool(name="cst", bufs=2) as cst,
            tc.tile_pool(name="wsb", bufs=24) as wsb,
            tc.tile_pool(name="xs", bufs=4) as xsp,
            tc.tile_pool(name="qt", bufs=8) as qtp,
            tc.tile_pool(name="va", bufs=16) as vap,
            tc.tile_pool(name="ctx", bufs=4) as ctxp,
            tc.tile_pool(name="exp", bufs=16) as expp,
            tc.tile_pool(name="wo", bufs=4) as wop,
            tc.tile_pool(name="fin", bufs=4) as finp,
            tc.tile_pool(name="rb", bufs=4) as rbp,
            tc.tile_pool(name="tmp", bufs=4) as tmpp,
            tc.tile_pool(name="psc", bufs=2, space="PSUM") as pscorep,
            tc.tile_pool(name="pcx", bufs=2, space="PSUM") as pctxp,
            tc.tile_pool(name="ppj", bufs=2, space="PSUM") as pprojp,
        ):
            qb_t = cst.tile([128, NM], F32, tag="cst")
            kb_t = cst.tile([128, NM], F32, tag="cst")
            sel_sb = cst.tile([68, G], BF, tag="sel", name="sel_sb")

            zero_fill = nc.gpsimd.to_reg(0.0)

            # Warm the ScalarE Exp table at kernel start: the table load that
            # precedes the first Exp does not reliably complete before the
            # first exp executes on a cold core, so trigger it long before
            # the real exps (twice, separated by the PE warm-up block).
            warm = cst.tile([1, 8], F32, tag="warm", name="warm")
            nc.vector.memset(warm[:, :], 0.0)
            nc.scalar.activation(warm[:, :], warm[:, :], AFT.Exp)

            # PE HAM warm-up: ~4us of dummy matmuls on a zeroed tile while
            # the initial DMAs land, so real matmuls start at K=8/8.
            wmt = cst.tile([128, 384], BF, tag="wmt", name="wmt")
            nc.vector.memset(wmt[:, :], 0.0)
            for wi in range(24):
                wps = pprojp.tile([128, 512], F32, tag="ppj", name=f"wps{wi}")
                nc.tensor.matmul(
                    wps[:, 0:256], wmt[:, 0:128], wmt[:, 128:384],
                    start=True, stop=True)
            nc.scalar.activation(warm[:, :], warm[:, :], AFT.Exp)

            # constant ones-slots pattern for v_aug cols [64..72) per head
            ones_c = cst.tile([128, 8 * 68], BF, tag="ones", name="ones_c")
            ones_c3 = ones_c[:, :].rearrange("p (h x) -> p h x", h=8)
            nc.vector.memset(ones_c3[:, :, 64:68], 0.0)
            for h in range(8):
                nc.vector.memset(
                    ones_c3[:, h : h + 1, 64 + (h % 4) : 65 + (h % 4)], 1.0)

            qT = [qtp.tile([128, S], BF, tag="qt", name=f"qT{m}") for m in range(NM)]
            kT = [qtp.tile([128, S], BF, tag="qt", name=f"kT{m}") for m in range(NM)]
            ctxT = [ctxp.tile([128, S], BF, tag="ctx", name=f"ctxT{m}")
                    for m in range(NM)]
            v_aug = [None] * NT

            nc.gpsimd.dma_start(qb_t[:, :], qb[:, :])
            nc.gpsimd.dma_start(kb_t[:, :], kb[:, :])
            nc.gpsimd.dma_start(sel_sb[64:68, :], sel[:, :])

            # weights stay resident for the whole kernel
            wq_sb = [wsb.tile([128, G], BF, tag="w", name=f"wq{kc}") for kc in range(KC)]
            wk_sb = [wsb.tile([128, G], BF, tag="w", name=f"wk{kc}") for kc in range(KC)]
            wv_sb = [wsb.tile([128, G], BF, tag="w", name=f"wv{kc}") for kc in range(KC)]
            for kc in range(KC):
                nc.gpsimd.dma_start(wq_sb[kc][:, :], wq[ts(kc, 128), :])
            for kc in range(KC):
                nc.gpsimd.dma_start(wk_sb[kc][:, :], wk[ts(kc, 128), :])
            for kc in range(KC):
                nc.gpsimd.dma_start(wv_sb[kc][:, :], wv[ts(kc, 128), :])
            wo_sb = [wop.tile([128, E], BF, tag="wo", name=f"wo{ec}") for ec in range(NM)]

            def proj_qk(n, x_ap, w_sb, dst, bias_t, scale, dma_eng):
                xsb = xsp.tile([128, KC * 512], BF, tag="xs", name="xsb")
                xs3 = xsb[:, :].rearrange("p (k c) -> p k c", k=KC)
                if n == 0:
                    # fine-grained loads, alternating HWDGE queues, so the
                    # first matmul chains are not DMA-trigger-paced
                    for kc in range(KC):
                        eng = dma_eng if kc % 2 == 0 else (
                            nc.scalar if dma_eng is nc.sync else nc.sync)
                        eng.dma_start(
                            xs3[:, kc, :], x_ap[ts(kc, 128), ts(n, 512)])
                else:
                    dma_eng.dma_start(
                        xs3[:, :, :],
                        x_ap[:, ts(n, 512)].rearrange("(k p) c -> p k c", p=128))
                xs = [xs3[:, kc, :] for kc in range(KC)]
                for mp in range(2):
                    ps0 = pprojp.tile([128, 512], F32, tag="ppj", name="pj0")
                    ps1 = pprojp.tile([128, 512], F32, tag="ppj", name="pj1")
                    for kc in range(KC):
                        nc.tensor.matmul(
                            ps0[:, :], w_sb[kc][:, ts(2 * mp, 128)], xs[kc],
                            start=(kc == 0), stop=(kc == KC - 1))
                        nc.tensor.matmul(
                            ps1[:, :], w_sb[kc][:, ts(2 * mp + 1, 128)], xs[kc],
                            start=(kc == 0), stop=(kc == KC - 1))
                    for mh, ps in ((0, ps0), (1, ps1)):
                        m = 2 * mp + mh
                        nc.vector.tensor_scalar(
                            dst[m][:, ts(n, 512)], ps[:, :],
                            scale, bias_t[:, m : m + 1],
                            ALU.mult, ALU.add)

            def proj_v(tg):
                xsb = xsp.tile([128, KC * 512], BF, tag="xs", name="xsvb")
                xs3 = xsb[:, :].rearrange("p (k c) -> p k c", k=KC)
                if tg == 0:
                    for kc in range(KC):
                        nc.sync.dma_start(
                            xs3[:, kc, :], xv[ts(kc, 128), ts(tg, 512)])
                else:
                    nc.sync.dma_start(
                        xs3[:, :, :],
                        xv[:, ts(tg, 512)].rearrange("(k p) c -> p k c", p=128))
                xs = [xs3[:, kc, :] for kc in range(KC)]
                for tp in range(2):
                    ps0 = pprojp.tile([128, 512], F32, tag="ppj", name="pv0")
                    ps1 = pprojp.tile([128, 512], F32, tag="ppj", name="pv1")
                    for kc in range(KC):
                        nc.tensor.matmul(
                            ps0[:, :], xs[kc][:, ts(2 * tp, 128)], wv_sb[kc][:, :],
                            start=(kc == 0), stop=(kc == KC - 1))
                        nc.tensor.matmul(
                            ps1[:, :], xs[kc][:, ts(2 * tp + 1, 128)], wv_sb[kc][:, :],
                            start=(kc == 0), stop=(kc == KC - 1))
                    for th, ps in ((0, ps0), (1, ps1)):
                        tt = 4 * tg + 2 * tp + th
                        va = vap.tile([128, 8 * 68], BF, tag="va", name=f"va{tt}")
                        va3 = va[:, :].rearrange("p (h x) -> p h x", h=8)
                        ps3 = ps[:, :].rearrange("p (h x) -> p h x", h=8)
                        nc.vector.tensor_copy(va3[:, :, 0:64], ps3[:, :, :])
                        nc.gpsimd.tensor_copy(
                            va3[:, :, 64:68], ones_c3[:, :, 64:68])
                        v_aug[tt] = va

            def att_unit(hp, j, rs_j, i_lo, i_hi, add_in):
                """Attention for k-tiles [i_lo, i_hi) of query block j,
                head-pair hp.  One score/exp tile per k-tile holds both
                heads ([A 512 | B 512]) so each exp frees a slot.
                add_in=False: overwrite ctxT (body / whole block);
                add_in=True: add to existing ctxT (diagonal part)."""
                cA = pctxp.tile([68, 512], F32, tag="pcx", name="cA")
                cB = pctxp.tile([68, 512], F32, tag="pcx", name="cB")
                hA, hB = 2 * hp, 2 * hp + 1
                for i in range(i_lo, i_hi):
                    r = i - 4 * j
                    c0 = 128 * r if r > 0 else 0   # valid q start in block
                    s = pscorep.tile([128, 1024], F32, tag="psc", name="s")
                    nc.tensor.matmul(
                        s[:, c0:512],
                        kT[hp][0:64, ts(i, 128)],
                        qT[hp][0:64, 512 * j + c0 : 512 * (j + 1)],
                        start=True, stop=True)
                    nc.tensor.matmul(
                        s[:, 512 + c0 : 1024],
                        kT[hp][64:128, ts(i, 128)],
                        qT[hp][64:128, 512 * j + c0 : 512 * (j + 1)],
                        start=True, stop=True, tile_position=(64, 0))
                    e = expp.tile([128, 1024], BF, tag="exp", name="e")
                    # diagonal tiles: cols [c0:512] (head A) and
                    # [512+c0:1024] (head B) are the only ranges read
                    # downstream — one contiguous exp over [c0:1024]
                    nc.scalar.activation(e[:, c0:1024], s[:, c0:1024], AFT.Exp)
                    if r >= 0:
                        for base in (0, 512):
                            # zero above-diagonal inside the 128-wide
                            # staircase band: keep iff col' - row >= 0
                            nc.gpsimd.affine_select(
                                out=e[:, base + 128 * r : base + 128 * (r + 1)],
                                in_=e[:, base + 128 * r : base + 128 * (r + 1)],
                                pattern=[[1, 128]],
                                compare_op=ALU.is_ge,
                                fill=zero_fill,
                                base=0,
                                channel_multiplier=-1)
                    nc.tensor.matmul(
                        cA[:, c0:512], v_aug[i][:, hA * 68 : hA * 68 + 68],
                        e[:, c0:512],
                        start=(i == i_lo), stop=(i == i_hi - 1))
                    nc.tensor.matmul(
                        cB[:, c0:512], v_aug[i][:, hB * 68 : hB * 68 + 68],
                        e[:, 512 + c0 : 1024],
                        start=(i == i_lo), stop=(i == i_hi - 1))
                # evacuate unnormalized ctx to SBUF; accumulate row-sums
                # (each head occupies its own partition in the 64..72 band)
                nc.vector.tensor_add(
                    rs_j[64:68, :], rs_j[64:68, :], cA[64:68, :])
                nc.vector.tensor_add(
                    rs_j[64:68, :], rs_j[64:68, :], cB[64:68, :])
                if not add_in:
                    nc.vector.tensor_copy(ctxT[hp][0:64, ts(j, 512)], cA[0:64, :])
                    tm = tmpp.tile([64, 512], BF, tag="tmp", name="tm")
                    nc.vector.tensor_copy(tm[:, :], cB[0:64, :])
                    nc.sync.dma_start(ctxT[hp][64:128, ts(j, 512)], tm[:, :])
                else:
                    nc.vector.tensor_add(
                        ctxT[hp][0:64, ts(j, 512)], ctxT[hp][0:64, ts(j, 512)],
                        cA[0:64, :])
                    tm = tmpp.tile([64, 512], BF, tag="tmp", name="tm")
                    nc.vector.tensor_copy(tm[:, :], cB[0:64, :])
                    sh = tmpp.tile([128, 512], BF, tag="tmp2", name="sh", bufs=2)
                    nc.sync.dma_start(sh[64:128, :], tm[:, :])
                    nc.vector.tensor_add(
                        ctxT[hp][64:128, ts(j, 512)], ctxT[hp][64:128, ts(j, 512)],
                        sh[64:128, :])

            def oproj_block(q4, ec_order=(0, 1, 2, 3)):
                for jtp in range(4):
                    ps0 = pprojp.tile([128, 512], F32, tag="ppj", name="po0")
                    ps1 = pprojp.tile([128, 512], F32, tag="ppj", name="po1")
                    for ei, ec in enumerate(ec_order):
                        nc.tensor.matmul(
                            ps0[:, :], wo_sb[ec][:, ts(2 * jtp, 128)],
                            ctxT[ec][:, ts(q4, 512)],
                            start=(ei == 0), stop=(ei == NM - 1))
                        nc.tensor.matmul(
                            ps1[:, :], wo_sb[ec][:, ts(2 * jtp + 1, 128)],
                            ctxT[ec][:, ts(q4, 512)],
                            start=(ei == 0), stop=(ei == NM - 1))
                    for oh, ps in ((0, ps0), (1, ps1)):
                        st = finp.tile([128, 512], BF, tag="fin", name="st")
                        nc.vector.tensor_copy(st[:, :], ps[:, :])
                        nc.sync.dma_start(
                            fT[128 * (2 * jtp + oh) : 128 * (2 * jtp + oh) + 128,
                               ts(q4, 512)],
                            st[:, :])

            def normalize_half(j, rs_x, hpp):
                # heads of hp = 2*hpp, 2*hpp+1
                rec = rbp.tile([68, 512], F32, tag="rec", name="rec", bufs=3)
                nc.vector.reciprocal(rec[64:68, :], rs_x[64:68, :])
                recb = rbp.tile([68, 512], BF, tag="recb", name="recb", bufs=3)
                nc.vector.tensor_copy(recb[64:68, :], rec[64:68, :])
                for hh in range(2):
                    hp = 2 * hpp + hh
                    psn = pprojp.tile([128, 512], F32, tag="ppj", name="psn")
                    nc.tensor.matmul(
                        psn[:, :],
                        sel_sb[64:68, ts(hp, 128)], recb[64:68, :],
                        start=True, stop=True, tile_position=(64, 0))
                    nc.vector.tensor_mul(
                        ctxT[hp][:, ts(j, 512)], ctxT[hp][:, ts(j, 512)],
                        psn[:, :])

            # ---- emission schedule -----------------------------------------
            proj_qk(0, xq, wq_sb, qT, qb_t, 0.125, nc.sync)
            proj_qk(0, xk, wk_sb, kT, kb_t, 1.0, nc.scalar)
            proj_v(0)
            # wo is only needed by the output projection; load late.
            for ec in range(NM):
                nc.sync.dma_start(wo_sb[ec][:, :], wo[ts(ec, 128), :])

            rs_ab = {}
            for j in range(NJ):
                rs_a = rbp.tile([68, 512], F32, tag="rs", name=f"rsa{j}", bufs=8)
                rs_b = rbp.tile([68, 512], F32, tag="rs", name=f"rsb{j}", bufs=8)
                rs_ab[j] = (rs_a, rs_b)

            def memset_rs(j):
                rs_a, rs_b = rs_ab[j]
                nc.vector.memset(rs_a[64:68, :], 0.0)
                nc.vector.memset(rs_b[64:68, :], 0.0)

            def att_part(j, i_lo, i_hi, add_in):
                rs_a, rs_b = rs_ab[j]
                att_unit(0, j, rs_a, i_lo, i_hi, add_in)
                att_unit(1, j, rs_a, i_lo, i_hi, add_in)
                att_unit(2, j, rs_b, i_lo, i_hi, add_in)
                att_unit(3, j, rs_b, i_lo, i_hi, add_in)

            def finish_block(j):
                rs_a, rs_b = rs_ab[j]
                normalize_half(j, rs_a, 0)
                normalize_half(j, rs_b, 1)
                oproj_block(j)

            def norm_only(j):
                rs_a, rs_b = rs_ab[j]
                normalize_half(j, rs_a, 0)
                normalize_half(j, rs_b, 1)

            # Filler placement: each projection / output-projection chunk is
            # emitted just before the exp-bound attention phase whose PE gaps
            # it should fill (priority ~ emission order).  oproj(0) is held
            # back for the final diagonal, which otherwise has no dense PE
            # work; oproj(3) necessarily lands in the tail.
            memset_rs(0)
            att_part(0, 0, 4, add_in=False)       # block 0 is diagonal-only
            proj_qk(1, xq, wq_sb, qT, qb_t, 0.125, nc.sync)
            norm_only(0)
            memset_rs(1)
            att_part(1, 0, 4, add_in=False)       # body(1): needs only qT(1)
            proj_qk(1, xk, wk_sb, kT, kb_t, 1.0, nc.scalar)
            proj_v(1)
            proj_qk(2, xq, wq_sb, qT, qb_t, 0.125, nc.sync)
            att_part(1, 4, 8, add_in=True)
            norm_only(1)
            memset_rs(2)
            att_part(2, 0, 8, add_in=False)
            proj_qk(2, xk, wk_sb, kT, kb_t, 1.0, nc.scalar)
            proj_v(2)
            oproj_block(1)
            att_part(2, 8, 12, add_in=True)
            proj_qk(3, xq, wq_sb, qT, qb_t, 0.125, nc.sync)
            norm_only(2)
            memset_rs(3)
            att_part(3, 0, 12, add_in=False)
            proj_qk(3, xk, wk_sb, kT, kb_t, 1.0, nc.scalar)
            proj_v(3)
            oproj_block(2)
            # tail: interleave the final diagonal with its normalization and
            # use oproj(0) as the dense PE filler for its exp-bound phase
            rs_a3, rs_b3 = rs_ab[3]
            att_unit(0, 3, rs_a3, 12, 16, add_in=True)
            att_unit(1, 3, rs_a3, 12, 16, add_in=True)
            oproj_block(0)
            normalize_half(3, rs_a3, 0)
            att_unit(2, 3, rs_b3, 12, 16, add_in=True)
            att_unit(3, 3, rs_b3, 12, 16, add_in=True)
            normalize_half(3, rs_b3, 1)
            oproj_block(3)

    nc.compile()
    return nc


def _get_nc():
    global _NC
    if _NC is None:
        _NC = _build()
    return _NC


def build_in_maps(inputs):
    query = np.asarray(inputs["query"], np.float32)
    key = np.asarray(inputs["key"], np.float32)
    value = np.asarray(inputs["value"], np.float32)
    q_w = np.asarray(inputs["q_w"], np.float32)
    q_b = np.asarray(inputs["q_b"], np.float32)
    k_w = np.asarray(inputs["k_w"], np.float32)
    k_b = np.asarray(inputs["k_b"], np.float32)
    v_w = np.asarray(inputs["v_w"], np.float32)
    o_w = np.asarray(inputs["o_w"], np.float32)

    xqT = [np.ascontiguousarray(query[b].T).astype(BF16) for b in range(B)]
    xkT = [np.ascontiguousarray(key[b].T).astype(BF16) for b in range(B)]
    xvT = [np.ascontiguousarray(value[b].T).astype(BF16) for b in range(B)]

    wqT, wkT, wvT, woT, qbt, kbt = [], [], [], [], [], []
    for g in range(2):
        gs = slice(g * G, (g + 1) * G)
        wqT.append(np.ascontiguousarray(q_w[gs, :].T).astype(BF16))
        wkT.append(np.ascontiguousarray(k_w[gs, :].T).astype(BF16))
        wvT.append(np.ascontiguousarray(v_w[gs, :].T).astype(BF16))
        woT.append(np.ascontiguousarray(o_w[:, gs].T).astype(BF16))
        qbt.append(
            np.ascontiguousarray((q_b[gs] / 8.0).reshape(NM, 128).T).astype(
                np.float32
            )
        )
        kbt.append(
            np.ascontiguousarray(k_b[gs].reshape(NM, 128).T).astype(np.float32)
        )

    sel_np = np.zeros((4, G), np.float32)
    for k in range(4):
        for p in range(G):
            hp, pp = p // 128, p % 128
            if k == (2 * hp + (pp // 64)) % 4:
                sel_np[k, p] = 1.0
    sel_np = sel_np.astype(BF16)

    in_maps = []
    for b in range(B):
        for g in range(2):
            in_maps.append(
                {
                    "xq": xqT[b],
                    "xk": xkT[b],
                    "xv": xvT[b],
                    "wq": wqT[g],
                    "wk": wkT[g],
                    "wv": wvT[g],
                    "wo": woT[g],
                    "qb": qbt[g],
                    "kb": kbt[g],
                    "sel": sel_np,
                }
            )

    return in_maps


def kernel(**inputs):
    nc = _get_nc()
    in_maps = build_in_maps(inputs)
    res = bass_utils.run_bass_kernel_spmd(nc, in_maps, core_ids=list(range(8)))

    o_b = np.asarray(inputs["o_b"], np.float32)
    v_b = np.asarray(inputs["v_b"], np.float32)
    o_w = np.asarray(inputs["o_w"], np.float32)
    corr = (o_b + v_b @ o_w.T).astype(np.float32)  # softmax rows sum to 1
    out = np.empty((B, S, E), np.float32)
    for b in range(B):
        acc = (res.results[2 * b]["fT"].astype(np.float32)
               + res.results[2 * b + 1]["fT"].astype(np.float32))
        out[b] = acc.T + corr[None, :]
    return out


# revision 27
# speedup vs baseline: 1.1208x; 1.1208x over previous
"""Multi-head attention (B=4, S=2048, E=1024, H=16, causal) on 8 Trainium2 cores.

Sharding: core = (batch b, head-group g) — 4 batches x 2 groups of 8 heads.
Each core computes q/k/v projections for its batch restricted to its 8 heads,
causal attention for those heads, and a partial output projection over its
512 ctx columns.  The host sums the two partials per batch and adds all
output-side bias terms analytically (softmax rows sum to one, so the v-bias
passes through attention unchanged: out += o_b + v_b @ o_w.T).

On-device layouts (per core):
  qT/kT: [head_dim 512 -> 4 tiles of 128, token 2048]  (2 heads per tile)
  v_aug: [token -> 16 tiles of 128, 8 heads x (64 dims + ones col)]
  scores are computed transposed (k^T q per head, contraction dim 64,
  two heads row-tiled concurrently in the PE array), softmax is max-free
  (scores are O(+-8), exp cannot overflow fp32), causal masking is
  structural: fully-masked tiles are skipped, diagonal tiles restrict the
  matmul N-range to the valid query span and a 128-wide affine_select
  staircase zeroes the in-band upper triangle.
  attn @ v is computed as v_aug^T @ expT giving ctx^T plus the softmax
  row-sum in one matmul (ones column of v_aug).

Scheduling structure (v3):
  The kernel is globally near-balanced (PE ~200us streaming, ScalarE exp
  ~180us), but causal attention concentrates exp work in the last query
  blocks.  To keep both engines busy everywhere — and to keep the PE duty
  cycle high enough that the HAM clock gate stays at full rate — each
  attention block j>=1 is split:
    body  = k-tiles 0..4j-1 (needs only qT(j) and earlier kT/v blocks)
    diag  = k-tiles 4j..4j+3 (needs kT/v of block j)
  All four q-projections run up front, so bodies of late blocks overlap
  the k/v projections of early blocks.  Body context sums are evacuated
  to SBUF (bf16) and the diagonal contribution is added in on the DVE.
  Dedicated PSUM pools (scores 2x[128,1024], ctx 2x[68,512],
  proj/oproj/norm 2x[128,512]) keep the phases from serializing on banks.
  Output fT is bf16 (halves the output DMA; host accumulates in f32).
"""

import os
import sys

for _p in ("/opt/trn_rl_repo", "/root/.axon_site/_ro/trn_rl_repo"):
    if os.path.isdir(_p) and _p not in sys.path:
        sys.path.append(_p)

import numpy as np
import ml_dtypes

import concourse.bacc as bacc
import concourse.mybir as mybir
from concourse import tile
from concourse import bass_utils
from concourse.bass import ts

BF16 = ml_dtypes.bfloat16
F32 = mybir.dt.float32
BF = mybir.dt.bfloat16
AFT = mybir.ActivationFunctionType
ALU = mybir.AluOpType

B, S, E = 4, 2048, 1024
H, D = 16, 64
G = 512            # head dims per core (8 heads)
KC = E // 128      # contraction chunks for projections
NM = G // 128      # m-tiles of the group dim
NJ = S // 512      # 512-wide token column blocks
NT = S // 128      # 128-wide token tiles

_NC = None


def _build():
    nc = bacc.Bacc("TRN2", target_bir_lowering=False, debug=False, num_devices=8)

    xq = nc.dram_tensor("xq", (E, S), BF, kind="ExternalInput").ap()
    xk = nc.dram_tensor("xk", (E, S), BF, kind="ExternalInput").ap()
    xv = nc.dram_tensor("xv", (E, S), BF, kind="ExternalInput").ap()
    wq = nc.dram_tensor("wq", (E, G), BF, kind="ExternalInput").ap()
    wk = nc.dram_tensor("wk", (E, G), BF, kind="ExternalInput").ap()
    wv = nc.dram_tensor("wv", (E, G), BF, kind="ExternalInput").ap()
    wo = nc.dram_tensor("wo", (G, E), BF, kind="ExternalInput").ap()
    qb = nc.dram_tensor("qb", (128, NM), F32, kind="ExternalInput").ap()
    kb = nc.dram_tensor("kb", (128, NM), F32, kind="ExternalInput").ap()
    sel = nc.dram_tensor("sel", (4, G), BF, kind="ExternalInput").ap()
    fT = nc.dram_tensor("fT", (E, S), BF, kind="ExternalOutput").ap()

    with tile.TileContext(nc) as tc:
        with (
            tc.tile_pool(name="cst", bufs=2) as cst,
            tc.tile_pool(name="wsb", bufs=24) as wsb,
            tc.tile_pool(name="xs", bufs=4) as xsp,
            tc.tile_pool(name="qt", bufs=8) as qtp,
            tc.tile_pool(name="va", bufs=16) as vap,
            tc.tile_pool(name="ctx", bufs=4) as ctxp,
            tc.tile_pool(name="exp", bufs=16) as expp,
            tc.tile_pool(name="wo", bufs=4) as wop,
            tc.tile_pool(name="fin", bufs=4) as finp,
            tc.tile_pool(name="rb", bufs=4) as rbp,
            tc.tile_pool(name="tmp", bufs=4) as tmpp,
            tc.tile_pool(name="psc", bufs=2, space="PSUM") as pscorep,
            tc.tile_pool(name="pcx", bufs=2, space="PSUM") as pctxp,
            tc.tile_pool(name="ppj", bufs=2, space="PSUM") as pprojp,
        ):
            qb_t = cst.tile([128, NM], F32, tag="cst")
            kb_t = cst.tile([128, NM], F32, tag="cst")
            sel_sb = cst.tile([68, G], BF, tag="sel", name="sel_sb")

            zero_fill = nc.gpsimd.to_reg(0.0)

            # Warm the ScalarE Exp table at kernel start: the table load that
            # precedes the first Exp does not reliably complete before the
            # first exp executes on a cold core, so trigger it long before
            # the real exps (twice, separated by the PE warm-up block).
            warm = cst.tile([1, 8], F32, tag="warm", name="warm")
            nc.vector.memset(warm[:, :], 0.0)
            nc.scalar.activation(warm[:, :], warm[:, :], AFT.Exp)

            # PE HAM warm-up: ~4us of dummy matmuls on a zeroed tile while
            # the initial DMAs land, so real matmuls start at K=8/8.
            wmt = cst.tile([128, 384], BF, tag="wmt", name="wmt")
            nc.vector.memset(wmt[:, :], 0.0)
            for wi in range(24):
                wps = pprojp.tile([128, 512], F32, tag="ppj", name=f"wps{wi}")
                nc.tensor.matmul(
                    wps[:, 0:256], wmt[:, 0:128], wmt[:, 128:384],
                    start=True, stop=True)
            nc.scalar.activation(warm[:, :], warm[:, :], AFT.Exp)

            # constant ones-slots pattern for v_aug cols [64..72) per head
            ones_c = cst.tile([128, 8 * 68], BF, tag="ones", name="ones_c")
            ones_c3 = ones_c[:, :].rearrange("p (h x) -> p h x", h=8)
            nc.vector.memset(ones_c3[:, :, 64:68], 0.0)
            for h in range(8):
                nc.vector.memset(
                    ones_c3[:, h : h + 1, 64 + (h % 4) : 65 + (h % 4)], 1.0)

            qT = [qtp.tile([128, S], BF, tag="qt", name=f"qT{m}") for m in range(NM)]
            kT = [qtp.tile([128, S], BF, tag="qt", name=f"kT{m}") for m in range(NM)]
            ctxT = [ctxp.tile([128, S], BF, tag="ctx", name=f"ctxT{m}")
                    for m in range(NM)]
            v_aug = [None] * NT

            nc.gpsimd.dma_start(qb_t[:, :], qb[:, :])
            nc.gpsimd.dma_start(kb_t[:, :], kb[:, :])
            nc.gpsimd.dma_start(sel_sb[64:68, :], sel[:, :])

            # weights stay resident for the whole kernel
            wq_sb = [wsb.tile([128, G], BF, tag="w", name=f"wq{kc}") for kc in range(KC)]
            wk_sb = [wsb.tile([128, G], BF, tag="w", name=f"wk{kc}") for kc in range(KC)]
            wv_sb = [wsb.tile([128, G], BF, tag="w", name=f"wv{kc}") for kc in range(KC)]
            for kc in range(KC):
                nc.gpsimd.dma_start(wq_sb[kc][:, :], wq[ts(kc, 128), :])
            for kc in range(KC):
                nc.gpsimd.dma_start(wk_sb[kc][:, :], wk[ts(kc, 128), :])
            for kc in range(KC):
                nc.gpsimd.dma_start(wv_sb[kc][:, :], wv[ts(kc, 128), :])
            wo_sb = [wop.tile([128, E], BF, tag="wo", name=f"wo{ec}") for ec in range(NM)]

            def proj_qk(n, x_ap, w_sb, dst, bias_t, scale, dma_eng):
                xsb = xsp.tile([128, KC * 512], BF, tag="xs", name="xsb")
                xs3 = xsb[:, :].rearrange("p (k c) -> p k c", k=KC)
                if n == 0:
                    # fine-grained loads, alternating HWDGE queues, so the
                    # first matmul chains are not DMA-trigger-paced
                    for kc in range(KC):
                        eng = dma_eng if kc % 2 == 0 else (
                            nc.scalar if dma_eng is nc.sync else nc.sync)
                        eng.dma_start(
                            xs3[:, kc, :], x_ap[ts(kc, 128), ts(n, 512)])
                else:
                    dma_eng.dma_start(
                        xs3[:, :, :],
                        x_ap[:, ts(n, 512)].rearrange("(k p) c -> p k c", p=128))
                xs = [xs3[:, kc, :] for kc in range(KC)]
                for mp in range(2):
                    ps0 = pprojp.tile([128, 512], F32, tag="ppj", name="pj0")
                    ps1 = pprojp.tile([128, 512], F32, tag="ppj", name="pj1")
                    for kc in range(KC):
                        nc.tensor.matmul(
                            ps0[:, :], w_sb[kc][:, ts(2 * mp, 128)], xs[kc],
                            start=(kc == 0), stop=(kc == KC - 1))
                        nc.tensor.matmul(
                            ps1[:, :], w_sb[kc][:, ts(2 * mp + 1, 128)], xs[kc],
                            start=(kc == 0), stop=(kc == KC - 1))
                    for mh, ps in ((0, ps0), (1, ps1)):
                        m = 2 * mp + mh
                        nc.vector.tensor_scalar(
                            dst[m][:, ts(n, 512)], ps[:, :],
                            scale, bias_t[:, m : m + 1],
                            ALU.mult, ALU.add)

            def proj_v(tg):
                xsb = xsp.tile([128, KC * 512], BF, tag="xs", name="xsvb")
                xs3 = xsb[:, :].rearrange("p (k c) -> p k c", k=KC)
                if tg == 0:
                    for kc in range(KC):
                        nc.sync.dma_start(
                            xs3[:, kc, :], xv[ts(kc, 128), ts(tg, 512)])
                else:
                    nc.sync.dma_start(
                        xs3[:, :, :],
                        xv[:, ts(tg, 512)].rearrange("(k p) c -> p k c", p=128))
                xs = [xs3[:, kc, :] for kc in range(KC)]
                for tp in range(2):
                    ps0 = pprojp.tile([128, 512], F32, tag="ppj", name="pv0")
                    ps1 = pprojp.tile([128, 512], F32, tag="ppj", name="pv1")
                    for kc in range(KC):
                        nc.tensor.matmul(
                            ps0[:, :], xs[kc][:, ts(2 * tp, 128)], wv_sb[kc][:, :],
                            start=(kc == 0), stop=(kc == KC - 1))
                        nc.tensor.matmul(
                            ps1[:, :], xs[kc][:, ts(2 * tp + 1, 128)], wv_sb[kc][:, :],
                            start=(kc == 0), stop=(kc == KC - 1))
                    for th, ps in ((0, ps0), (1, ps1)):
                        tt = 4 * tg + 2 * tp + th
                        va = vap.tile([128, 8 * 68], BF, tag="va", name=f"va{tt}")
                        va3 = va[:, :].rearrange("p (h x) -> p h x", h=8)
                        ps3 = ps[:, :].rearrange("p (h x) -> p h x", h=8)
                        nc.vector.tensor_copy(va3[:, :, 0:64], ps3[:, :, :])
                        nc.gpsimd.tensor_copy(
                            va3[:, :, 64:68], ones_c3[:, :, 64:68])
                        v_aug[tt] = va

            def att_unit(hp, j, rs_j, i_lo, i_hi, add_in):
                """Attention for k-tiles [i_lo, i_hi) of query block j,
                head-pair hp.  add_in=False: overwrite ctxT (body / whole
                block); add_in=True: add to existing ctxT (diagonal part)."""
                cA = pctxp.tile([68, 512], F32, tag="pcx", name="cA")
                cB = pctxp.tile([68, 512], F32, tag="pcx", name="cB")
                hA, hB = 2 * hp, 2 * hp + 1
                for ip in range(i_lo // 2, i_hi // 2):
                    i0, i1 = 2 * ip, 2 * ip + 1
                    sA = pscorep.tile([128, 1024], F32, tag="psc", name="sA")
                    sB = pscorep.tile([128, 1024], F32, tag="psc", name="sB")
                    for half, i in ((0, i0), (1, i1)):
                        r = i - 4 * j
                        c0 = 128 * r if r > 0 else 0   # valid q start in block
                        nc.tensor.matmul(
                            sA[:, 512 * half + c0 : 512 * half + 512],
                            kT[hp][0:64, ts(i, 128)],
                            qT[hp][0:64, 512 * j + c0 : 512 * (j + 1)],
                            start=True, stop=True)
                        nc.tensor.matmul(
                            sB[:, 512 * half + c0 : 512 * half + 512],
                            kT[hp][64:128, ts(i, 128)],
                            qT[hp][64:128, 512 * j + c0 : 512 * (j + 1)],
                            start=True, stop=True, tile_position=(64, 0))
                    eA = expp.tile([128, 1024], BF, tag="exp", name="eA")
                    eB = expp.tile([128, 1024], BF, tag="exp", name="eB")
                    # second diagonal pair: only cols [256:1024] are ever
                    # read downstream — restrict the exp width
                    if i0 - 4 * j == 2:
                        nc.scalar.activation(eA[:, 256:1024], sA[:, 256:1024],
                                             AFT.Exp)
                        nc.scalar.activation(eB[:, 256:1024], sB[:, 256:1024],
                                             AFT.Exp)
                    else:
                        nc.scalar.activation(eA[:, :], sA[:, :], AFT.Exp)
                        nc.scalar.activation(eB[:, :], sB[:, :], AFT.Exp)
                    for half, i in ((0, i0), (1, i1)):
                        r = i - 4 * j
                        if r < 0:
                            continue
                        for e in (eA, eB):
                            # zero above-diagonal inside the 128-wide
                            # staircase band: keep iff col' - row >= 0
                            nc.gpsimd.affine_select(
                                out=e[:, 512 * half + 128 * r : 512 * half + 128 * (r + 1)],
                                in_=e[:, 512 * half + 128 * r : 512 * half + 128 * (r + 1)],
                                pattern=[[1, 128]],
                                compare_op=ALU.is_ge,
                                fill=zero_fill,
                                base=0,
                                channel_multiplier=-1)
                    for half, i in ((0, i0), (1, i1)):
                        r = i - 4 * j
                        c0 = 128 * r if r > 0 else 0
                        nc.tensor.matmul(
                            cA[:, c0:512], v_aug[i][:, hA * 68 : hA * 68 + 68],
                            eA[:, 512 * half + c0 : 512 * half + 512],
                            start=(i == i_lo), stop=(i == i_hi - 1))
                        nc.tensor.matmul(
                            cB[:, c0:512], v_aug[i][:, hB * 68 : hB * 68 + 68],
                            eB[:, 512 * half + c0 : 512 * half + 512],
                            start=(i == i_lo), stop=(i == i_hi - 1))
                # evacuate unnormalized ctx to SBUF; accumulate row-sums
                # (each head occupies its own partition in the 64..68 band)
                nc.vector.tensor_add(
                    rs_j[64:68, :], rs_j[64:68, :], cA[64:68, :])
                nc.vector.tensor_add(
                    rs_j[64:68, :], rs_j[64:68, :], cB[64:68, :])
                if not add_in:
                    nc.vector.tensor_copy(ctxT[hp][0:64, ts(j, 512)], cA[0:64, :])
                    tm = tmpp.tile([64, 512], BF, tag="tmp", name="tm")
                    nc.vector.tensor_copy(tm[:, :], cB[0:64, :])
                    nc.sync.dma_start(ctxT[hp][64:128, ts(j, 512)], tm[:, :])
                else:
                    nc.vector.tensor_add(
                        ctxT[hp][0:64, ts(j, 512)], ctxT[hp][0:64, ts(j, 512)],
                        cA[0:64, :])
                    tm = tmpp.tile([64, 512], BF, tag="tmp", name="tm")
                    nc.vector.tensor_copy(tm[:, :], cB[0:64, :])
                    sh = tmpp.tile([128, 512], BF, tag="tmp2", name="sh", bufs=2)
                    nc.sync.dma_start(sh[64:128, :], tm[:, :])
                    nc.vector.tensor_add(
                        ctxT[hp][64:128, ts(j, 512)], ctxT[hp][64:128, ts(j, 512)],
                        sh[64:128, :])

            def oproj_block(q4, ec_order=(0, 1, 2, 3)):
                for jtp in range(4):
                    ps0 = pprojp.tile([128, 512], F32, tag="ppj", name="po0")
                    ps1 = pprojp.tile([128, 512], F32, tag="ppj", name="po1")
                    for ei, ec in enumerate(ec_order):
                        nc.tensor.matmul(
                            ps0[:, :], wo_sb[ec][:, ts(2 * jtp, 128)],
                            ctxT[ec][:, ts(q4, 512)],
                            start=(ei == 0), stop=(ei == NM - 1))
                        nc.tensor.matmul(
                            ps1[:, :], wo_sb[ec][:, ts(2 * jtp + 1, 128)],
                            ctxT[ec][:, ts(q4, 512)],
                            start=(ei == 0), stop=(ei == NM - 1))
                    for oh, ps in ((0, ps0), (1, ps1)):
                        st = finp.tile([128, 512], BF, tag="fin", name="st")
                        nc.vector.tensor_copy(st[:, :], ps[:, :])
                        nc.sync.dma_start(
                            fT[128 * (2 * jtp + oh) : 128 * (2 * jtp + oh) + 128,
                               ts(q4, 512)],
                            st[:, :])

            def normalize_split(j, rs_x, hp, rec, recb):
                # tail variant: per-pair reciprocal so it overlaps the other
                # pair's attention.  rec/recb are shared across the group so
                # the second (unaligned-base) pair can matmul over [64:68].
                r0 = 64 + 2 * (hp % 2)
                psn = pprojp.tile([128, 512], F32, tag="ppj", name="psn")
                if r0 == 64:
                    nc.vector.reciprocal(rec[64:66, :], rs_x[64:66, :])
                    nc.vector.tensor_copy(recb[64:66, :], rec[64:66, :])
                    nc.tensor.matmul(
                        psn[:, :],
                        sel_sb[64:66, ts(hp, 128)], recb[64:66, :],
                        start=True, stop=True, tile_position=(64, 0))
                else:
                    # recompute rows 64:66 too (same values — aligned base);
                    # sel is zero there for this pair's columns anyway
                    nc.vector.reciprocal(rec[64:68, :], rs_x[64:68, :])
                    nc.vector.tensor_copy(recb[64:68, :], rec[64:68, :])
                    nc.tensor.matmul(
                        psn[:, :],
                        sel_sb[64:68, ts(hp, 128)], recb[64:68, :],
                        start=True, stop=True, tile_position=(64, 0))
                nc.vector.tensor_mul(
                    ctxT[hp][:, ts(j, 512)], ctxT[hp][:, ts(j, 512)],
                    psn[:, :])

            def normalize_half(j, rs_x, hpp):
                # one [4,512] reciprocal covers both pairs of the group
                rec = rbp.tile([68, 512], F32, tag="rec", name="rec", bufs=3)
                nc.vector.reciprocal(rec[64:68, :], rs_x[64:68, :])
                recb = rbp.tile([68, 512], BF, tag="recb", name="recb", bufs=3)
                nc.vector.tensor_copy(recb[64:68, :], rec[64:68, :])
                for hh in range(2):
                    hp = 2 * hpp + hh
                    psn = pprojp.tile([128, 512], F32, tag="ppj", name="psn")
                    nc.tensor.matmul(
                        psn[:, :],
                        sel_sb[64:68, ts(hp, 128)], recb[64:68, :],
                        start=True, stop=True, tile_position=(64, 0))
                    nc.vector.tensor_mul(
                        ctxT[hp][:, ts(j, 512)], ctxT[hp][:, ts(j, 512)],
                        psn[:, :])

            # ---- emission schedule -----------------------------------------
            proj_qk(0, xq, wq_sb, qT, qb_t, 0.125, nc.sync)
            proj_qk(0, xk, wk_sb, kT, kb_t, 1.0, nc.scalar)
            proj_v(0)
            # wo is only needed by the output projection; load late.
            for ec in range(NM):
                nc.sync.dma_start(wo_sb[ec][:, :], wo[ts(ec, 128), :])

            rs_ab = {}
            for j in range(NJ):
                rs_a = rbp.tile([68, 512], F32, tag="rs", name=f"rsa{j}", bufs=8)
                rs_b = rbp.tile([68, 512], F32, tag="rs", name=f"rsb{j}", bufs=8)
                rs_ab[j] = (rs_a, rs_b)

            def memset_rs(j):
                rs_a, rs_b = rs_ab[j]
                nc.vector.memset(rs_a[64:68, :], 0.0)
                nc.vector.memset(rs_b[64:68, :], 0.0)

            def att_part(j, i_lo, i_hi, add_in):
                rs_a, rs_b = rs_ab[j]
                att_unit(0, j, rs_a, i_lo, i_hi, add_in)
                att_unit(1, j, rs_a, i_lo, i_hi, add_in)
                att_unit(2, j, rs_b, i_lo, i_hi, add_in)
                att_unit(3, j, rs_b, i_lo, i_hi, add_in)

            def finish_block(j):
                rs_a, rs_b = rs_ab[j]
                normalize_half(j, rs_a, 0)
                normalize_half(j, rs_b, 1)
                oproj_block(j)

            def norm_only(j):
                rs_a, rs_b = rs_ab[j]
                normalize_half(j, rs_a, 0)
                normalize_half(j, rs_b, 1)

            # Filler placement: each projection / output-projection chunk is
            # emitted just before the exp-bound attention phase whose PE gaps
            # it should fill (priority ~ emission order).  oproj(0) is held
            # back for the final diagonal, which otherwise has no dense PE
            # work; oproj(3) necessarily lands in the tail.
            memset_rs(0)
            att_part(0, 0, 4, add_in=False)       # block 0 is diagonal-only
            proj_qk(1, xq, wq_sb, qT, qb_t, 0.125, nc.sync)
            norm_only(0)
            memset_rs(1)
            att_part(1, 0, 4, add_in=False)       # body(1): needs only qT(1)
            proj_qk(1, xk, wk_sb, kT, kb_t, 1.0, nc.scalar)
            proj_v(1)
            proj_qk(2, xq, wq_sb, qT, qb_t, 0.125, nc.sync)
            att_part(1, 4, 8, add_in=True)
            norm_only(1)
            memset_rs(2)
            att_part(2, 0, 8, add_in=False)
            proj_qk(2, xk, wk_sb, kT, kb_t, 1.0, nc.scalar)
            proj_v(2)
            oproj_block(1)
            att_part(2, 8, 12, add_in=True)
            proj_qk(3, xq, wq_sb, qT, qb_t, 0.125, nc.sync)
            norm_only(2)
            memset_rs(3)
            att_part(3, 0, 12, add_in=False)
            proj_qk(3, xk, wk_sb, kT, kb_t, 1.0, nc.scalar)
            proj_v(3)
            oproj_block(2)
            # tail: interleave the final diagonal with its normalization and
            # use oproj(0) as the dense PE filler for its exp-bound phase
            rs_a3, rs_b3 = rs_ab[3]
            att_unit(0, 3, rs_a3, 12, 16, add_in=True)
            att_unit(1, 3, rs_a3, 12, 16, add_in=True)
            oproj_block(0)
            normalize_half(3, rs_a3, 0)
            rec3 = rbp.tile([68, 512], F32, tag="rec", name="rec3", bufs=3)
            recb3 = rbp.tile([68, 512], BF, tag="recb", name="recb3", bufs=3)
            att_unit(2, 3, rs_b3, 12, 16, add_in=True)
            normalize_split(3, rs_b3, 2, rec3, recb3)
            att_unit(3, 3, rs_b3, 12, 16, add_in=True)
            normalize_split(3, rs_b3, 3, rec3, recb3)
            oproj_block(3)

    nc.compile()
    return nc


def _get_nc():
    global _NC
    if _NC is None:
        _NC = _build()
    return _NC


def build_in_maps(inputs):
    query = np.asarray(inputs["query"], np.float32)
    key = np.asarray(inputs["key"], np.float32)
    value = np.asarray(inputs["value"], np.float32)
    q_w = np.asarray(inputs["q_w"], np.float32)
    q_b = np.asarray(inputs["q_b"], np.float32)
    k_w = np.asarray(inputs["k_w"], np.float32)
    k_b = np.asarray(inputs["k_b"], np.float32)
    v_w = np.asarray(inputs["v_w"], np.float32)
    o_w = np.asarray(inputs["o_w"], np.float32)

    xqT = [np.ascontiguousarray(query[b].T).astype(BF16) for b in range(B)]
    xkT = [np.ascontiguousarray(key[b].T).astype(BF16) for b in range(B)]
    xvT = [np.ascontiguousarray(value[b].T).astype(BF16) for b in range(B)]

    wqT, wkT, wvT, woT, qbt, kbt = [], [], [], [], [], []
    for g in range(2):
        gs = slice(g * G, (g + 1) * G)
        wqT.append(np.ascontiguousarray(q_w[gs, :].T).astype(BF16))
        wkT.append(np.ascontiguousarray(k_w[gs, :].T).astype(BF16))
        wvT.append(np.ascontiguousarray(v_w[gs, :].T).astype(BF16))
        woT.append(np.ascontiguousarray(o_w[:, gs].T).astype(BF16))
        qbt.append(
            np.ascontiguousarray((q_b[gs] / 8.0).reshape(NM, 128).T).astype(
                np.float32
            )
        )
        kbt.append(
            np.ascontiguousarray(k_b[gs].reshape(NM, 128).T).astype(np.float32)
        )

    sel_np = np.zeros((4, G), np.float32)
    for k in range(4):
        for p in range(G):
            hp, pp = p // 128, p % 128
            if k == (2 * hp + (pp // 64)) % 4:
                sel_np[k, p] = 1.0
    sel_np = sel_np.astype(BF16)

    in_maps = []
    for b in range(B):
        for g in range(2):
            in_maps.append(
                {
                    "xq": xqT[b],
                    "xk": xkT[b],
                    "xv": xvT[b],
                    "wq": wqT[g],
                    "wk": wkT[g],
                    "wv": wvT[g],
                    "wo": woT[g],
                    "qb": qbt[g],
                    "kb": kbt[g],
                    "sel": sel_np,
                }
            )

    return in_maps


def kernel(**inputs):
    nc = _get_nc()
    in_maps = build_in_maps(inputs)
    res = bass_utils.run_bass_kernel_spmd(nc, in_maps, core_ids=list(range(8)))

    o_b = np.asarray(inputs["o_b"], np.float32)
    v_b = np.asarray(inputs["v_b"], np.float32)
    o_w = np.asarray(inputs["o_w"], np.float32)
    corr = (o_b + v_b @ o_w.T).astype(np.float32)  # softmax rows sum to 1
    out = np.empty((B, S, E), np.float32)
    for b in range(B):
        acc = (res.results[2 * b]["fT"].astype(np.float32)
               + res.results[2 * b + 1]["fT"].astype(np.float32))
        out[b] = acc.T + corr[None, :]
    return out


# revision 28
# speedup vs baseline: 1.1549x; 1.0305x over previous
"""Multi-head attention (B=4, S=2048, E=1024, H=16, causal) on 8 Trainium2 cores.

Sharding: core = (batch b, head-group g) — 4 batches x 2 groups of 8 heads.
Each core computes q/k/v projections for its batch restricted to its 8 heads,
causal attention for those heads, and a partial output projection over its
512 ctx columns.  The host sums the two partials per batch and adds all
output-side bias terms analytically (softmax rows sum to one, so the v-bias
passes through attention unchanged: out += o_b + v_b @ o_w.T).

On-device layouts (per core):
  qT/kT: [head_dim 512 -> 4 tiles of 128, token 2048]  (2 heads per tile)
  v_aug: [token -> 16 tiles of 128, 8 heads x (64 dims + ones col)]
  scores are computed transposed (k^T q per head, contraction dim 64,
  two heads row-tiled concurrently in the PE array), softmax is max-free
  (scores are O(+-8), exp cannot overflow fp32), causal masking is
  structural: fully-masked tiles are skipped, diagonal tiles restrict the
  matmul N-range to the valid query span and a 128-wide affine_select
  staircase zeroes the in-band upper triangle.
  attn @ v is computed as v_aug^T @ expT giving ctx^T plus the softmax
  row-sum in one matmul (ones column of v_aug).

Scheduling structure (v3):
  The kernel is globally near-balanced (PE ~200us streaming, ScalarE exp
  ~180us), but causal attention concentrates exp work in the last query
  blocks.  To keep both engines busy everywhere — and to keep the PE duty
  cycle high enough that the HAM clock gate stays at full rate — each
  attention block j>=1 is split:
    body  = k-tiles 0..4j-1 (needs only qT(j) and earlier kT/v blocks)
    diag  = k-tiles 4j..4j+3 (needs kT/v of block j)
  All four q-projections run up front, so bodies of late blocks overlap
  the k/v projections of early blocks.  Body context sums are evacuated
  to SBUF (bf16) and the diagonal contribution is added in on the DVE.
  Dedicated PSUM pools (scores 2x[128,1024], ctx 2x[68,512],
  proj/oproj/norm 2x[128,512]) keep the phases from serializing on banks.
  Output fT is bf16 (halves the output DMA; host accumulates in f32).
"""

import os
import sys

for _p in ("/opt/trn_rl_repo", "/root/.axon_site/_ro/trn_rl_repo"):
    if os.path.isdir(_p) and _p not in sys.path:
        sys.path.append(_p)

import numpy as np
import ml_dtypes

import concourse.bacc as bacc
import concourse.mybir as mybir
from concourse import tile
from concourse import bass_utils
from concourse.bass import ts

BF16 = ml_dtypes.bfloat16
F32 = mybir.dt.float32
BF = mybir.dt.bfloat16
AFT = mybir.ActivationFunctionType
ALU = mybir.AluOpType

B, S, E = 4, 2048, 1024
H, D = 16, 64
G = 512            # head dims per core (8 heads)
KC = E // 128      # contraction chunks for projections
NM = G // 128      # m-tiles of the group dim
NJ = S // 512      # 512-wide token column blocks
NT = S // 128      # 128-wide token tiles

_NC = None


def _build():
    nc = bacc.Bacc("TRN2", target_bir_lowering=False, debug=False, num_devices=8)

    xq = nc.dram_tensor("xq", (E, S), BF, kind="ExternalInput").ap()
    xk = nc.dram_tensor("xk", (E, S), BF, kind="ExternalInput").ap()
    xv = nc.dram_tensor("xv", (E, S), BF, kind="ExternalInput").ap()
    wq = nc.dram_tensor("wq", (E, G), BF, kind="ExternalInput").ap()
    wk = nc.dram_tensor("wk", (E, G), BF, kind="ExternalInput").ap()
    wv = nc.dram_tensor("wv", (E, G), BF, kind="ExternalInput").ap()
    wo = nc.dram_tensor("wo", (G, E), BF, kind="ExternalInput").ap()
    qb = nc.dram_tensor("qb", (128, NM), F32, kind="ExternalInput").ap()
    kb = nc.dram_tensor("kb", (128, NM), F32, kind="ExternalInput").ap()
    sel = nc.dram_tensor("sel", (4, G), BF, kind="ExternalInput").ap()
    fT = nc.dram_tensor("fT", (E, S), BF, kind="ExternalOutput").ap()

    with tile.TileContext(nc) as tc:
        with (
            tc.tile_pool(name="cst", bufs=2) as cst,
            tc.tile_pool(name="wsb", bufs=24) as wsb,
            tc.tile_pool(name="xs", bufs=6) as xsp,
            tc.tile_pool(name="qt", bufs=8) as qtp,
            tc.tile_pool(name="va", bufs=16) as vap,
            tc.tile_pool(name="ctx", bufs=4) as ctxp,
            tc.tile_pool(name="exp", bufs=10) as expp,
            tc.tile_pool(name="wo", bufs=4) as wop,
            tc.tile_pool(name="fin", bufs=4) as finp,
            tc.tile_pool(name="rb", bufs=4) as rbp,
            tc.tile_pool(name="tmp", bufs=4) as tmpp,
            tc.tile_pool(name="psc", bufs=2, space="PSUM") as pscorep,
            tc.tile_pool(name="pcx", bufs=2, space="PSUM") as pctxp,
            tc.tile_pool(name="ppj", bufs=2, space="PSUM") as pprojp,
        ):
            qb_t = cst.tile([128, NM], F32, tag="cst")
            kb_t = cst.tile([128, NM], F32, tag="cst")
            sel_sb = cst.tile([68, G], BF, tag="sel", name="sel_sb")

            zero_fill = nc.gpsimd.to_reg(0.0)

            # Warm the ScalarE Exp table at kernel start: the table load that
            # precedes the first Exp does not reliably complete before the
            # first exp executes on a cold core, so trigger it long before
            # the real exps (twice, separated by the PE warm-up block).
            warm = cst.tile([1, 8], F32, tag="warm", name="warm")
            nc.vector.memset(warm[:, :], 0.0)
            nc.scalar.activation(warm[:, :], warm[:, :], AFT.Exp)

            # PE HAM warm-up: ~4us of dummy matmuls on a zeroed tile while
            # the initial DMAs land, so real matmuls start at K=8/8.
            wmt = cst.tile([128, 384], BF, tag="wmt", name="wmt")
            nc.vector.memset(wmt[:, :], 0.0)
            for wi in range(24):
                wps = pprojp.tile([128, 512], F32, tag="ppj", name=f"wps{wi}")
                nc.tensor.matmul(
                    wps[:, 0:256], wmt[:, 0:128], wmt[:, 128:384],
                    start=True, stop=True)
            nc.scalar.activation(warm[:, :], warm[:, :], AFT.Exp)

            # constant ones-slots pattern for v_aug cols [64..72) per head
            ones_c = cst.tile([128, 8 * 68], BF, tag="ones", name="ones_c")
            ones_c3 = ones_c[:, :].rearrange("p (h x) -> p h x", h=8)
            nc.vector.memset(ones_c3[:, :, 64:68], 0.0)
            for h in range(8):
                nc.vector.memset(
                    ones_c3[:, h : h + 1, 64 + (h % 4) : 65 + (h % 4)], 1.0)

            qT = [qtp.tile([128, S], BF, tag="qt", name=f"qT{m}") for m in range(NM)]
            kT = [qtp.tile([128, S], BF, tag="qt", name=f"kT{m}") for m in range(NM)]
            ctxT = [ctxp.tile([128, S], BF, tag="ctx", name=f"ctxT{m}")
                    for m in range(NM)]
            v_aug = [None] * NT

            nc.gpsimd.dma_start(qb_t[:, :], qb[:, :])
            nc.gpsimd.dma_start(kb_t[:, :], kb[:, :])
            nc.gpsimd.dma_start(sel_sb[64:68, :], sel[:, :])

            # weights stay resident for the whole kernel
            wq_sb = [wsb.tile([128, G], BF, tag="w", name=f"wq{kc}") for kc in range(KC)]
            wk_sb = [wsb.tile([128, G], BF, tag="w", name=f"wk{kc}") for kc in range(KC)]
            wv_sb = [wsb.tile([128, G], BF, tag="w", name=f"wv{kc}") for kc in range(KC)]
            for kc in range(KC):
                nc.gpsimd.dma_start(wq_sb[kc][:, :], wq[ts(kc, 128), :])
            for kc in range(KC):
                nc.gpsimd.dma_start(wk_sb[kc][:, :], wk[ts(kc, 128), :])
            for kc in range(KC):
                nc.gpsimd.dma_start(wv_sb[kc][:, :], wv[ts(kc, 128), :])
            wo_sb = [wop.tile([128, E], BF, tag="wo", name=f"wo{ec}") for ec in range(NM)]

            def proj_qk(n, x_ap, w_sb, dst, bias_t, scale, dma_eng):
                xsb = xsp.tile([128, KC * 512], BF, tag="xs", name="xsb")
                xs3 = xsb[:, :].rearrange("p (k c) -> p k c", k=KC)
                if n == 0:
                    # fine-grained loads, alternating HWDGE queues, so the
                    # first matmul chains are not DMA-trigger-paced
                    for kc in range(KC):
                        eng = dma_eng if kc % 2 == 0 else (
                            nc.scalar if dma_eng is nc.sync else nc.sync)
                        eng.dma_start(
                            xs3[:, kc, :], x_ap[ts(kc, 128), ts(n, 512)])
                else:
                    dma_eng.dma_start(
                        xs3[:, :, :],
                        x_ap[:, ts(n, 512)].rearrange("(k p) c -> p k c", p=128))
                xs = [xs3[:, kc, :] for kc in range(KC)]
                for mp in range(2):
                    ps0 = pprojp.tile([128, 512], F32, tag="ppj", name="pj0")
                    ps1 = pprojp.tile([128, 512], F32, tag="ppj", name="pj1")
                    for kc in range(KC):
                        nc.tensor.matmul(
                            ps0[:, :], w_sb[kc][:, ts(2 * mp, 128)], xs[kc],
                            start=(kc == 0), stop=(kc == KC - 1))
                        nc.tensor.matmul(
                            ps1[:, :], w_sb[kc][:, ts(2 * mp + 1, 128)], xs[kc],
                            start=(kc == 0), stop=(kc == KC - 1))
                    for mh, ps in ((0, ps0), (1, ps1)):
                        m = 2 * mp + mh
                        nc.vector.tensor_scalar(
                            dst[m][:, ts(n, 512)], ps[:, :],
                            scale, bias_t[:, m : m + 1],
                            ALU.mult, ALU.add)

            def proj_v(tg):
                xsb = xsp.tile([128, KC * 512], BF, tag="xs", name="xsvb")
                xs3 = xsb[:, :].rearrange("p (k c) -> p k c", k=KC)
                if tg == 0:
                    for kc in range(KC):
                        nc.sync.dma_start(
                            xs3[:, kc, :], xv[ts(kc, 128), ts(tg, 512)])
                else:
                    nc.sync.dma_start(
                        xs3[:, :, :],
                        xv[:, ts(tg, 512)].rearrange("(k p) c -> p k c", p=128))
                xs = [xs3[:, kc, :] for kc in range(KC)]
                for tp in range(2):
                    ps0 = pprojp.tile([128, 512], F32, tag="ppj", name="pv0")
                    ps1 = pprojp.tile([128, 512], F32, tag="ppj", name="pv1")
                    for kc in range(KC):
                        nc.tensor.matmul(
                            ps0[:, :], xs[kc][:, ts(2 * tp, 128)], wv_sb[kc][:, :],
                            start=(kc == 0), stop=(kc == KC - 1))
                        nc.tensor.matmul(
                            ps1[:, :], xs[kc][:, ts(2 * tp + 1, 128)], wv_sb[kc][:, :],
                            start=(kc == 0), stop=(kc == KC - 1))
                    for th, ps in ((0, ps0), (1, ps1)):
                        tt = 4 * tg + 2 * tp + th
                        va = vap.tile([128, 8 * 68], BF, tag="va", name=f"va{tt}")
                        va3 = va[:, :].rearrange("p (h x) -> p h x", h=8)
                        ps3 = ps[:, :].rearrange("p (h x) -> p h x", h=8)
                        nc.vector.tensor_copy(va3[:, :, 0:64], ps3[:, :, :])
                        nc.gpsimd.tensor_copy(
                            va3[:, :, 64:68], ones_c3[:, :, 64:68])
                        v_aug[tt] = va

            def att_unit(hp, j, rs_j, i_lo, i_hi, add_in):
                """Attention for k-tiles [i_lo, i_hi) of query block j,
                head-pair hp.  add_in=False: overwrite ctxT (body / whole
                block); add_in=True: add to existing ctxT (diagonal part)."""
                cA = pctxp.tile([68, 512], F32, tag="pcx", name="cA")
                cB = pctxp.tile([68, 512], F32, tag="pcx", name="cB")
                hA, hB = 2 * hp, 2 * hp + 1
                for ip in range(i_lo // 2, i_hi // 2):
                    i0, i1 = 2 * ip, 2 * ip + 1
                    sA = pscorep.tile([128, 1024], F32, tag="psc", name="sA")
                    sB = pscorep.tile([128, 1024], F32, tag="psc", name="sB")
                    for half, i in ((0, i0), (1, i1)):
                        r = i - 4 * j
                        c0 = 128 * r if r > 0 else 0   # valid q start in block
                        nc.tensor.matmul(
                            sA[:, 512 * half + c0 : 512 * half + 512],
                            kT[hp][0:64, ts(i, 128)],
                            qT[hp][0:64, 512 * j + c0 : 512 * (j + 1)],
                            start=True, stop=True)
                        nc.tensor.matmul(
                            sB[:, 512 * half + c0 : 512 * half + 512],
                            kT[hp][64:128, ts(i, 128)],
                            qT[hp][64:128, 512 * j + c0 : 512 * (j + 1)],
                            start=True, stop=True, tile_position=(64, 0))
                    eA = expp.tile([128, 1024], BF, tag="exp", name="eA")
                    eB = expp.tile([128, 1024], BF, tag="exp", name="eB")
                    # second diagonal pair: only cols [256:1024] are ever
                    # read downstream — restrict the exp width
                    if i0 - 4 * j == 2:
                        nc.scalar.activation(eA[:, 256:1024], sA[:, 256:1024],
                                             AFT.Exp)
                        nc.scalar.activation(eB[:, 256:1024], sB[:, 256:1024],
                                             AFT.Exp)
                    else:
                        nc.scalar.activation(eA[:, :], sA[:, :], AFT.Exp)
                        nc.scalar.activation(eB[:, :], sB[:, :], AFT.Exp)
                    for half, i in ((0, i0), (1, i1)):
                        r = i - 4 * j
                        if r < 0:
                            continue
                        for e in (eA, eB):
                            # zero above-diagonal inside the 128-wide
                            # staircase band: keep iff col' - row >= 0
                            nc.gpsimd.affine_select(
                                out=e[:, 512 * half + 128 * r : 512 * half + 128 * (r + 1)],
                                in_=e[:, 512 * half + 128 * r : 512 * half + 128 * (r + 1)],
                                pattern=[[1, 128]],
                                compare_op=ALU.is_ge,
                                fill=zero_fill,
                                base=0,
                                channel_multiplier=-1)
                    for half, i in ((0, i0), (1, i1)):
                        r = i - 4 * j
                        c0 = 128 * r if r > 0 else 0
                        nc.tensor.matmul(
                            cA[:, c0:512], v_aug[i][:, hA * 68 : hA * 68 + 68],
                            eA[:, 512 * half + c0 : 512 * half + 512],
                            start=(i == i_lo), stop=(i == i_hi - 1))
                        nc.tensor.matmul(
                            cB[:, c0:512], v_aug[i][:, hB * 68 : hB * 68 + 68],
                            eB[:, 512 * half + c0 : 512 * half + 512],
                            start=(i == i_lo), stop=(i == i_hi - 1))
                # evacuate unnormalized ctx to SBUF; accumulate row-sums
                # (each head occupies its own partition in the 64..68 band)
                nc.vector.tensor_add(
                    rs_j[64:68, :], rs_j[64:68, :], cA[64:68, :])
                nc.vector.tensor_add(
                    rs_j[64:68, :], rs_j[64:68, :], cB[64:68, :])
                if not add_in:
                    nc.vector.tensor_copy(ctxT[hp][0:64, ts(j, 512)], cA[0:64, :])
                    tm = tmpp.tile([64, 512], BF, tag="tmp", name="tm")
                    nc.vector.tensor_copy(tm[:, :], cB[0:64, :])
                    nc.sync.dma_start(ctxT[hp][64:128, ts(j, 512)], tm[:, :])
                else:
                    nc.vector.tensor_add(
                        ctxT[hp][0:64, ts(j, 512)], ctxT[hp][0:64, ts(j, 512)],
                        cA[0:64, :])
                    tm = tmpp.tile([64, 512], BF, tag="tmp", name="tm")
                    nc.vector.tensor_copy(tm[:, :], cB[0:64, :])
                    sh = tmpp.tile([128, 512], BF, tag="tmp2", name="sh", bufs=2)
                    nc.sync.dma_start(sh[64:128, :], tm[:, :])
                    nc.vector.tensor_add(
                        ctxT[hp][64:128, ts(j, 512)], ctxT[hp][64:128, ts(j, 512)],
                        sh[64:128, :])

            def oproj_block(q4, ec_order=(0, 1, 2, 3)):
                for jtp in range(4):
                    ps0 = pprojp.tile([128, 512], F32, tag="ppj", name="po0")
                    ps1 = pprojp.tile([128, 512], F32, tag="ppj", name="po1")
                    for ei, ec in enumerate(ec_order):
                        nc.tensor.matmul(
                            ps0[:, :], wo_sb[ec][:, ts(2 * jtp, 128)],
                            ctxT[ec][:, ts(q4, 512)],
                            start=(ei == 0), stop=(ei == NM - 1))
                        nc.tensor.matmul(
                            ps1[:, :], wo_sb[ec][:, ts(2 * jtp + 1, 128)],
                            ctxT[ec][:, ts(q4, 512)],
                            start=(ei == 0), stop=(ei == NM - 1))
                    for oh, ps in ((0, ps0), (1, ps1)):
                        st = finp.tile([128, 512], BF, tag="fin", name="st")
                        nc.vector.tensor_copy(st[:, :], ps[:, :])
                        nc.sync.dma_start(
                            fT[128 * (2 * jtp + oh) : 128 * (2 * jtp + oh) + 128,
                               ts(q4, 512)],
                            st[:, :])

            def normalize_split(j, rs_x, hp, rec, recb):
                # tail variant: per-pair reciprocal so it overlaps the other
                # pair's attention.  rec/recb are shared across the group so
                # the second (unaligned-base) pair can matmul over [64:68].
                r0 = 64 + 2 * (hp % 2)
                psn = pprojp.tile([128, 512], F32, tag="ppj", name="psn")
                if r0 == 64:
                    nc.vector.reciprocal(rec[64:66, :], rs_x[64:66, :])
                    nc.vector.tensor_copy(recb[64:66, :], rec[64:66, :])
                    nc.tensor.matmul(
                        psn[:, :],
                        sel_sb[64:66, ts(hp, 128)], recb[64:66, :],
                        start=True, stop=True, tile_position=(64, 0))
                else:
                    # recompute rows 64:66 too (same values — aligned base);
                    # sel is zero there for this pair's columns anyway
                    nc.vector.reciprocal(rec[64:68, :], rs_x[64:68, :])
                    nc.vector.tensor_copy(recb[64:68, :], rec[64:68, :])
                    nc.tensor.matmul(
                        psn[:, :],
                        sel_sb[64:68, ts(hp, 128)], recb[64:68, :],
                        start=True, stop=True, tile_position=(64, 0))
                nc.vector.tensor_mul(
                    ctxT[hp][:, ts(j, 512)], ctxT[hp][:, ts(j, 512)],
                    psn[:, :])

            def normalize_half(j, rs_x, hpp):
                # one [4,512] reciprocal covers both pairs of the group
                rec = rbp.tile([68, 512], F32, tag="rec", name="rec", bufs=3)
                nc.vector.reciprocal(rec[64:68, :], rs_x[64:68, :])
                recb = rbp.tile([68, 512], BF, tag="recb", name="recb", bufs=3)
                nc.vector.tensor_copy(recb[64:68, :], rec[64:68, :])
                for hh in range(2):
                    hp = 2 * hpp + hh
                    psn = pprojp.tile([128, 512], F32, tag="ppj", name="psn")
                    nc.tensor.matmul(
                        psn[:, :],
                        sel_sb[64:68, ts(hp, 128)], recb[64:68, :],
                        start=True, stop=True, tile_position=(64, 0))
                    nc.vector.tensor_mul(
                        ctxT[hp][:, ts(j, 512)], ctxT[hp][:, ts(j, 512)],
                        psn[:, :])

            # ---- emission schedule -----------------------------------------
            proj_qk(0, xq, wq_sb, qT, qb_t, 0.125, nc.sync)
            proj_qk(0, xk, wk_sb, kT, kb_t, 1.0, nc.scalar)
            proj_v(0)
            # wo is only needed by the output projection; load late.
            for ec in range(NM):
                nc.sync.dma_start(wo_sb[ec][:, :], wo[ts(ec, 128), :])

            rs_ab = {}
            for j in range(NJ):
                rs_a = rbp.tile([68, 512], F32, tag="rs", name=f"rsa{j}", bufs=8)
                rs_b = rbp.tile([68, 512], F32, tag="rs", name=f"rsb{j}", bufs=8)
                rs_ab[j] = (rs_a, rs_b)

            def memset_rs(j):
                rs_a, rs_b = rs_ab[j]
                nc.vector.memset(rs_a[64:68, :], 0.0)
                nc.vector.memset(rs_b[64:68, :], 0.0)

            def att_part(j, i_lo, i_hi, add_in):
                rs_a, rs_b = rs_ab[j]
                att_unit(0, j, rs_a, i_lo, i_hi, add_in)
                att_unit(1, j, rs_a, i_lo, i_hi, add_in)
                att_unit(2, j, rs_b, i_lo, i_hi, add_in)
                att_unit(3, j, rs_b, i_lo, i_hi, add_in)

            def finish_block(j):
                rs_a, rs_b = rs_ab[j]
                normalize_half(j, rs_a, 0)
                normalize_half(j, rs_b, 1)
                oproj_block(j)

            def norm_only(j):
                rs_a, rs_b = rs_ab[j]
                normalize_half(j, rs_a, 0)
                normalize_half(j, rs_b, 1)

            # Filler placement: each projection / output-projection chunk is
            # emitted just before the exp-bound attention phase whose PE gaps
            # it should fill (priority ~ emission order).  oproj(0) is held
            # back for the final diagonal, which otherwise has no dense PE
            # work; oproj(3) necessarily lands in the tail.
            memset_rs(0)
            att_part(0, 0, 4, add_in=False)       # block 0 is diagonal-only
            proj_qk(1, xq, wq_sb, qT, qb_t, 0.125, nc.sync)
            norm_only(0)
            memset_rs(1)
            att_part(1, 0, 4, add_in=False)       # body(1): needs only qT(1)
            proj_qk(1, xk, wk_sb, kT, kb_t, 1.0, nc.scalar)
            proj_v(1)
            proj_qk(2, xq, wq_sb, qT, qb_t, 0.125, nc.sync)
            att_part(1, 4, 8, add_in=True)
            norm_only(1)
            memset_rs(2)
            att_part(2, 0, 8, add_in=False)
            proj_qk(2, xk, wk_sb, kT, kb_t, 1.0, nc.scalar)
            proj_v(2)
            oproj_block(1)
            att_part(2, 8, 12, add_in=True)
            proj_qk(3, xq, wq_sb, qT, qb_t, 0.125, nc.sync)
            norm_only(2)
            memset_rs(3)
            att_part(3, 0, 12, add_in=False)
            proj_qk(3, xk, wk_sb, kT, kb_t, 1.0, nc.scalar)
            proj_v(3)
            oproj_block(2)
            # tail: interleave the final diagonal with its normalization and
            # use oproj(0) as the dense PE filler for its exp-bound phase
            rs_a3, rs_b3 = rs_ab[3]
            att_unit(0, 3, rs_a3, 12, 16, add_in=True)
            att_unit(1, 3, rs_a3, 12, 16, add_in=True)
            oproj_block(0)
            normalize_half(3, rs_a3, 0)
            rec3 = rbp.tile([68, 512], F32, tag="rec", name="rec3", bufs=3)
            recb3 = rbp.tile([68, 512], BF, tag="recb", name="recb3", bufs=3)
            att_unit(2, 3, rs_b3, 12, 16, add_in=True)
            normalize_split(3, rs_b3, 2, rec3, recb3)
            att_unit(3, 3, rs_b3, 12, 16, add_in=True)
            normalize_split(3, rs_b3, 3, rec3, recb3)
            oproj_block(3)

    nc.compile()
    return nc


def _get_nc():
    global _NC
    if _NC is None:
        _NC = _build()
    return _NC


def build_in_maps(inputs):
    query = np.asarray(inputs["query"], np.float32)
    key = np.asarray(inputs["key"], np.float32)
    value = np.asarray(inputs["value"], np.float32)
    q_w = np.asarray(inputs["q_w"], np.float32)
    q_b = np.asarray(inputs["q_b"], np.float32)
    k_w = np.asarray(inputs["k_w"], np.float32)
    k_b = np.asarray(inputs["k_b"], np.float32)
    v_w = np.asarray(inputs["v_w"], np.float32)
    o_w = np.asarray(inputs["o_w"], np.float32)

    xqT = [np.ascontiguousarray(query[b].T).astype(BF16) for b in range(B)]
    xkT = [np.ascontiguousarray(key[b].T).astype(BF16) for b in range(B)]
    xvT = [np.ascontiguousarray(value[b].T).astype(BF16) for b in range(B)]

    wqT, wkT, wvT, woT, qbt, kbt = [], [], [], [], [], []
    for g in range(2):
        gs = slice(g * G, (g + 1) * G)
        wqT.append(np.ascontiguousarray(q_w[gs, :].T).astype(BF16))
        wkT.append(np.ascontiguousarray(k_w[gs, :].T).astype(BF16))
        wvT.append(np.ascontiguousarray(v_w[gs, :].T).astype(BF16))
        woT.append(np.ascontiguousarray(o_w[:, gs].T).astype(BF16))
        qbt.append(
            np.ascontiguousarray((q_b[gs] / 8.0).reshape(NM, 128).T).astype(
                np.float32
            )
        )
        kbt.append(
            np.ascontiguousarray(k_b[gs].reshape(NM, 128).T).astype(np.float32)
        )

    sel_np = np.zeros((4, G), np.float32)
    for k in range(4):
        for p in range(G):
            hp, pp = p // 128, p % 128
            if k == (2 * hp + (pp // 64)) % 4:
                sel_np[k, p] = 1.0
    sel_np = sel_np.astype(BF16)

    in_maps = []
    for b in range(B):
        for g in range(2):
            in_maps.append(
                {
                    "xq": xqT[b],
                    "xk": xkT[b],
                    "xv": xvT[b],
                    "wq": wqT[g],
                    "wk": wkT[g],
                    "wv": wvT[g],
                    "wo": woT[g],
                    "qb": qbt[g],
                    "kb": kbt[g],
                    "sel": sel_np,
                }
            )

    return in_maps


def kernel(**inputs):
    nc = _get_nc()
    in_maps = build_in_maps(inputs)
    res = bass_utils.run_bass_kernel_spmd(nc, in_maps, core_ids=list(range(8)))

    o_b = np.asarray(inputs["o_b"], np.float32)
    v_b = np.asarray(inputs["v_b"], np.float32)
    o_w = np.asarray(inputs["o_w"], np.float32)
    corr = (o_b + v_b @ o_w.T).astype(np.float32)  # softmax rows sum to 1
    out = np.empty((B, S, E), np.float32)
    for b in range(B):
        acc = (res.results[2 * b]["fT"].astype(np.float32)
               + res.results[2 * b + 1]["fT"].astype(np.float32))
        out[b] = acc.T + corr[None, :]
    return out
